# revision 21
# baseline (speedup 1.0000x reference)
"""CRF loss kernel for Trainium2 (8 NeuronCores, time-sharded).

Math: the log-domain forward recurrence
    alpha_t[i] = logsumexp_j(alpha_{t-1}[j] + trans[i,j]) + feat_t[i]
is run in probability domain:
    P_t = exp(feat_t - c) * (E @ P_{t-1}),   E = exp(trans)
so each step is one matmul plus one VectorE multiply.

Sharding: the per-step op cost is dominated by fixed per-instruction
overheads (125ns DVE PSUM-access bubble, ~100ns matmul result latency), so
batch width is nearly free and the 513 serial steps are the wall. E and
D_t = diag(exp(feat)) are strictly positive, so the normalized state
direction contracts to the true one in a handful of steps (measured: 1e-5
direction error after 8 steps; even 1 step leaves the end-to-end error at
the bf16 noise floor). Each core therefore owns a 64-step time block over
ALL 512 batch columns, warm-starting 1 step early from a uniform state;
core 0 starts exactly from p0. The host telescopes per-block log-norm
growth factors (column sums of the archived state at local steps 1 and
65, computed host-side in f64) to recover the exact log-partition value
at each column's capture slot seq_len+1. 65 steps never leave bf16 range
from a norm-1 start, so no mid-block renormalization is needed at all.

Layout: T=64 tags use half the 128 SBUF partitions, so two 256-column
groups are stacked on the partition axis (block-diagonal 128x128
transition matrix); per local step the state is [128, 256] split into 2
interleaved chains of 128 free columns. Steady state is DVE-bound at
~517ns/step = 2 x (125ns PSUM bubble + 128x1.04ns). All features ship
pre-exponentiated (host exp costs nothing and keeps the Act engine off
the critical path), the first 26 steps split across boot DMAs on three
parallel DGE queues (SP/Act/Pool) so the chain starts immediately. The
full [128, 256] state history archives straight from the history buffer
(DMA cost is free-size only, so shipping all partitions costs the same
as the two STOP rows), streamed out on all three DGE queues with a small
final block so the post-chain drain is one DMA latency.
"""
import numpy as np

_B, _S, _T = 512, 512, 64
_NCORE = 8
_P = 128
_START, _STOP = 62, 63
_WARM = 1                    # warmup steps (direction contraction)
_RLOC = 65                   # local steps per core
_FREE = 256                  # free columns per step tile (512 cols / 2)
_CW = _FREE // 2             # 128 free columns per chain
_NBOOT = 26                  # locals 1.._NBOOT ship via boot DMAs
_NB1 = 10                    # b0f: 1..2; b1: 3..10; b2a: 11..18; b2b: 19..26
_NB2A = 18

_cache = {}


def _build_nc():
    import concourse.bass as bass
    import concourse.bacc as bacc
    import concourse.tile as tile
    from concourse import mybir
    from contextlib import ExitStack

    f32 = mybir.dt.float32
    bf16 = mybir.dt.bfloat16
    nc = bacc.Bacc("TRN2", target_bir_lowering=False, debug=False,
                   num_devices=_NCORE)
    # bootE: [E2T | p0]
    bcols = _P + _FREE
    bootE = nc.dram_tensor("bootE", [_P, bcols], bf16,
                           kind="ExternalInput").ap()
    b0f = nc.dram_tensor("b0f", [_P, 3 * _FREE], bf16,
                         kind="ExternalInput").ap()
    bootb1 = nc.dram_tensor("bootb1", [_P, (_NB1 - 3) * _FREE], bf16,
                            kind="ExternalInput").ap()
    bootb2a = nc.dram_tensor("bootb2a", [_P, (_NB2A - _NB1) * _FREE], bf16,
                             kind="ExternalInput").ap()
    bootb2b = nc.dram_tensor("bootb2b", [_P, (_NBOOT - _NB2A) * _FREE], bf16,
                             kind="ExternalInput").ap()
    # pre-exp'd features for locals _NBOOT+1.._RLOC
    featR = nc.dram_tensor("featR", [_P, (_RLOC - _NBOOT) * _FREE], bf16,
                           kind="ExternalInput").ap()
    histo = nc.dram_tensor("hist", [_P, (_RLOC + 1) * _FREE], bf16,
                           kind="ExternalOutput").ap()

    with tile.TileContext(nc) as tc, ExitStack() as ctx:
        consts = ctx.enter_context(tc.tile_pool(name="consts", bufs=1))
        fpool = ctx.enter_context(tc.tile_pool(name="fpool", bufs=3))
        ps_g = [ctx.enter_context(
            tc.tile_pool(name=f"ps{g}", bufs=3, space="PSUM"))
            for g in range(2)]
        bootE_sb = consts.tile([_P, bcols], bf16, name="bootE_sb")
        nc.sync.dma_start(bootE_sb[:, :], bootE)            # SP first
        b0f_sb = consts.tile([_P, 3 * _FREE], bf16, name="b0f_sb")
        nc.scalar.dma_start(b0f_sb[:, :], b0f)              # Act queue
        b1_sb = consts.tile([_P, (_NB1 - 3) * _FREE], bf16, name="b1_sb")
        nc.gpsimd.dma_start(b1_sb[:, :], bootb1)            # Pool queue
        b2a_sb = consts.tile([_P, (_NB2A - _NB1) * _FREE], bf16,
                             name="b2a_sb")
        nc.sync.dma_start(b2a_sb[:, :], bootb2a)            # SP second
        b2b_sb = consts.tile([_P, (_NBOOT - _NB2A) * _FREE], bf16,
                             name="b2b_sb")
        nc.sync.dma_start(b2b_sb[:, :], bootb2b)            # SP third

        E2_sb = bootE_sb[:, 0:_P]
        p0 = bootE_sb[:, _P:_P + _FREE]

        hist = consts.tile([_P, (_RLOC + 1) * _FREE], bf16, name="hist")

        chunks = [(27, 34), (35, 50), (51, 65)]
        chunk_iter = iter(chunks)
        state = {"l0": None, "e": None}

        def fsrc(l):
            """F tile (exp'd, bf16) for local step l, from boot or chunk."""
            if l <= 3:
                return b0f_sb[:, (l - 1) * _FREE:l * _FREE]
            if l <= _NB1:
                off = (l - 4) * _FREE
                return b1_sb[:, off:off + _FREE]
            if l <= _NB2A:
                off = (l - _NB1 - 1) * _FREE
                return b2a_sb[:, off:off + _FREE]
            if l <= _NBOOT:
                off = (l - _NB2A - 1) * _FREE
                return b2b_sb[:, off:off + _FREE]
            off = (l - state["l0"]) * _FREE
            return state["e"][:, off:off + _FREE]

        next_chunk = next(chunk_iter)
        curs = [p0[:, g * _CW:(g + 1) * _CW] for g in range(2)]
        # archive blocks: last_slot -> (engine, first_slot)
        arch_plan = {24: ('pool', 1), 48: ('act', 25), 58: ('sp', 49),
                     63: ('pool', 59), 65: ('act', 64)}

        for l in range(1, _RLOC + 1):
            if next_chunk is not None and l == next_chunk[0]:
                lo, hi = next_chunk
                n = hi - lo + 1
                fch = fpool.tile([_P, 16 * _FREE], bf16, tag="fch")
                nc.sync.dma_start(fch[:, :n * _FREE],
                                  featR[:, (lo - _NBOOT - 1) * _FREE:
                                        (hi - _NBOOT) * _FREE])
                state["l0"], state["e"] = lo, fch
                next_chunk = next(chunk_iter, None)
            f_l = fsrc(l)
            for g in range(2):
                fsl = f_l[:, g * _CW:(g + 1) * _CW]
                ps = ps_g[g].tile([_P, _CW], f32, tag=f"ps{g}")
                nc.tensor.matmul(ps[:, :], E2_sb, curs[g],
                                 start=True, stop=True)
                dst = hist[:, l * _FREE + g * _CW:
                           l * _FREE + (g + 1) * _CW]
                nc.vector.tensor_mul(dst, ps[:, :], fsl)
                curs[g] = dst
            if l in arch_plan:
                eng_name, lo = arch_plan[l]
                eng = {'pool': nc.gpsimd, 'act': nc.scalar,
                       'sp': nc.sync}[eng_name]
                eng.dma_start(histo[:, lo * _FREE:(l + 1) * _FREE],
                              hist[:, lo * _FREE:(l + 1) * _FREE])
    nc.compile()
    return nc


def _prep_inputs(feas, transitions):
    import ml_dtypes
    bf = ml_dtypes.bfloat16

    E = np.exp(transitions.astype(np.float32))
    rows = np.ones(_T, bool)
    rows[_START] = False
    c = float(np.log(E.sum(1)[rows]).mean())
    ET = np.ascontiguousarray(E.T).astype(np.float32)       # ET[j,i]=E[i,j]
    E2T = np.zeros((_P, _P), np.float32)
    E2T[:_T, :_T] = ET
    E2T[_T:, _T:] = ET
    # stacked raw features per local step: stk[p, l, n] =
    #   feat[p%64, base+l-1, (p//64)*256 + n] - c   (pad -c past S)
    ft = np.transpose(feas.astype(np.float32), (2, 1, 0)) - np.float32(c)
    # ft: [T, S, B]
    in_maps = []
    for cix in range(_NCORE):
        base = 64 * cix
        stk = np.full((_P, _RLOC + 1, _FREE), -c, np.float32)
        n_real = min(_S - base, _RLOC)              # locals with real feats
        sl = ft[:, base:base + n_real, :]           # [T, n, B]
        stk[:_T, 1:n_real + 1, :] = np.ascontiguousarray(
            sl[:, :, 0:_FREE])
        stk[_T:, 1:n_real + 1, :] = np.ascontiguousarray(
            sl[:, :, _FREE:2 * _FREE])
        expF = np.exp(stk.astype(bf).astype(np.float32)).astype(bf)
        if cix == 0:
            p0 = np.zeros((_P, _FREE), np.float32)
            p0[_START, :] = 1.0
            p0[_T + _START, :] = 1.0
        else:
            p0 = np.full((_P, _FREE), 1.0 / _T, np.float32)
        bootE = np.hstack([E2T, p0]).astype(bf)
        in_maps.append({
            "bootE": np.ascontiguousarray(bootE),
            "b0f": np.ascontiguousarray(
                expF[:, 1:4, :].reshape(_P, -1)),
            "bootb1": np.ascontiguousarray(
                expF[:, 4:_NB1 + 1, :].reshape(_P, -1)),
            "bootb2a": np.ascontiguousarray(
                expF[:, _NB1 + 1:_NB2A + 1, :].reshape(_P, -1)),
            "bootb2b": np.ascontiguousarray(
                expF[:, _NB2A + 1:_NBOOT + 1, :].reshape(_P, -1)),
            "featR": np.ascontiguousarray(
                expF[:, _NBOOT + 1:, :].reshape(_P, -1)),
        })
    return c, in_maps


def kernel(feas, transitions, tag, seq_len):
    from concourse.bass_utils import run_bass_kernel_spmd

    feas = np.asarray(feas)
    transitions = np.asarray(transitions)
    tag = np.asarray(tag)
    seq_len = np.asarray(seq_len)

    if "nc" not in _cache:
        _cache["nc"] = _build_nc()
    nc = _cache["nc"]

    c, in_maps = _prep_inputs(feas, transitions)
    res = run_bass_kernel_spmd(nc, in_maps, list(range(_NCORE))).results

    # ---- host epilogue: telescoped norm from per-core archives ----
    L = seq_len.astype(np.int64)                                      # [B]
    # stops[j, l, b]: archived STOP value; col b -> (row 63 | 127, n=b%256)
    stops = np.zeros((_NCORE, _RLOC + 1, _B))
    s_start = np.zeros((_NCORE, _B))
    s_end = np.zeros((_NCORE, _B))
    for j in range(_NCORE):
        h = res[j]["hist"].reshape(_P, _RLOC + 1, _FREE).astype(np.float64)
        stops[j, :, 0:_FREE] = h[_STOP]
        stops[j, :, _FREE:2 * _FREE] = h[_T + _STOP]
        s_start[j, 0:_FREE] = h[0:_T, _WARM, :].sum(0)
        s_start[j, _FREE:] = h[_T:, _WARM, :].sum(0)
        s_end[j, 0:_FREE] = h[0:_T, _RLOC, :].sum(0)
        s_end[j, _FREE:] = h[_T:, _RLOC, :].sum(0)

    # block growth: core j covers global steps (64j+WARM, 64j+RLOC]
    growth = np.log(s_end) - np.log(s_start)            # [NCORE, B]
    growth[0] = np.log(s_end[0])                        # core 0: from |p0|=1
    prefix = np.concatenate([np.zeros((1, _B)), np.cumsum(growth, 0)], 0)

    m = L + 1                                           # capture slot
    K = np.where(m <= _RLOC, 0, (m - _RLOC - 1) // 64 + 1)
    lloc = m - 64 * K
    bb = np.arange(_B)
    C_raw = np.log(stops[K, lloc, bb])
    lvalue = np.where(
        K == 0,
        C_raw,
        C_raw + prefix[K, bb] - np.log(s_start[K, bb]),
    )
    featT_val = np.where(
        L < _S,
        feas[bb, np.minimum(L, _S - 1), _STOP].astype(np.float64) - c,
        -c,
    )
    norm = c * L + lvalue - featT_val

    # ---- gold score ----
    dt = np.float32
    pos = np.arange(_S + 2)
    lbl = np.concatenate(
        [np.full((_B, 1), _START, tag.dtype), tag,
         np.full((_B, 1), _STOP, tag.dtype)], axis=1,
    )
    lbl = np.where(pos[None, :] <= L[:, None], lbl, _STOP)
    trn = transitions[lbl[:, 1:], lbl[:, :-1]]
    tmask = (np.arange(_S + 1)[None, :] <= L[:, None]).astype(dt)
    trans_score = (trn.astype(dt) * tmask).sum(1)
    emit = np.take_along_axis(feas, tag[..., None], axis=2)[..., 0]
    emask = (np.arange(_S)[None, :] < L[:, None]).astype(dt)
    emit_score = (emit.astype(dt) * emask).sum(1)

    return (norm - (trans_score + emit_score)).astype(np.float32)


# revision 22
# speedup vs baseline: 1.0026x; 1.0026x over previous
"""CRF loss kernel for Trainium2 (8 NeuronCores, time-sharded).

Math: the log-domain forward recurrence
    alpha_t[i] = logsumexp_j(alpha_{t-1}[j] + trans[i,j]) + feat_t[i]
is run in probability domain:
    P_t = exp(feat_t - c) * (E @ P_{t-1}),   E = exp(trans)
so each step is one matmul plus one VectorE multiply.

Sharding: the per-step op cost is dominated by fixed per-instruction
overheads (125ns DVE PSUM-access bubble, ~100ns matmul result latency), so
batch width is nearly free and the 513 serial steps are the wall. E and
D_t = diag(exp(feat)) are strictly positive, so the normalized state
direction contracts to the true one in a handful of steps (measured: 1e-5
direction error after 8 steps; even 1 step leaves the end-to-end error at
the bf16 noise floor). Each core therefore owns a 64-step time block over
ALL 512 batch columns, warm-starting 1 step early from a uniform state;
core 0 starts exactly from p0. The host telescopes per-block log-norm
growth factors (column sums of the archived state at local steps 1 and
65, computed host-side in f64) to recover the exact log-partition value
at each column's capture slot seq_len+1. 65 steps never leave bf16 range
from a norm-1 start, so no mid-block renormalization is needed at all.

Layout: T=64 tags use half the 128 SBUF partitions, so two 256-column
groups are stacked on the partition axis (block-diagonal 128x128
transition matrix); per local step the state is [128, 256] split into 2
interleaved chains of 128 free columns. Steady state is DVE-bound at
~517ns/step = 2 x (125ns PSUM bubble + 128x1.04ns). All features ship
pre-exponentiated (host exp costs nothing and keeps the Act engine off
the critical path), the first 26 steps split across boot DMAs on three
parallel DGE queues (SP/Act/Pool) so the chain starts immediately. The
full [128, 256] state history archives straight from the history buffer
(DMA cost is free-size only, so shipping all partitions costs the same
as the two STOP rows), streamed out on all three DGE queues with a small
final block so the post-chain drain is one DMA latency.
"""
import numpy as np

_B, _S, _T = 512, 512, 64
_NCORE = 8
_P = 128
_START, _STOP = 62, 63
_WARM = 1                    # warmup steps (direction contraction)
_RLOC = 65                   # local steps per core
_FREE = 256                  # free columns per step tile (512 cols / 2)
_CW = _FREE // 2             # 128 free columns per chain
_NBOOT = 26                  # locals 1.._NBOOT ship via boot DMAs
_NB1 = 10                    # b0f: 1..2; b1: 3..10; b2a: 11..18; b2b: 19..26
_NB2A = 18

_cache = {}


def _build_nc():
    import concourse.bass as bass
    import concourse.bacc as bacc
    import concourse.tile as tile
    from concourse import mybir
    from contextlib import ExitStack

    f32 = mybir.dt.float32
    bf16 = mybir.dt.bfloat16
    nc = bacc.Bacc("TRN2", target_bir_lowering=False, debug=False,
                   num_devices=_NCORE)
    # bootE: [E2T | p0]
    bcols = _P + _FREE
    bootE = nc.dram_tensor("bootE", [_P, bcols], bf16,
                           kind="ExternalInput").ap()
    b0f = nc.dram_tensor("b0f", [_P, 3 * _FREE], bf16,
                         kind="ExternalInput").ap()
    bootb1 = nc.dram_tensor("bootb1", [_P, (_NB1 - 3) * _FREE], bf16,
                            kind="ExternalInput").ap()
    bootb2a = nc.dram_tensor("bootb2a", [_P, (_NB2A - _NB1) * _FREE], bf16,
                             kind="ExternalInput").ap()
    bootb2b = nc.dram_tensor("bootb2b", [_P, (_NBOOT - _NB2A) * _FREE], bf16,
                             kind="ExternalInput").ap()
    # pre-exp'd features for locals _NBOOT+1.._RLOC
    featR = nc.dram_tensor("featR", [_P, (_RLOC - _NBOOT) * _FREE], bf16,
                           kind="ExternalInput").ap()
    histo = nc.dram_tensor("hist", [_P, (_RLOC + 1) * _FREE], bf16,
                           kind="ExternalOutput").ap()

    with tile.TileContext(nc) as tc, ExitStack() as ctx:
        consts = ctx.enter_context(tc.tile_pool(name="consts", bufs=1))
        ps_g = [ctx.enter_context(
            tc.tile_pool(name=f"ps{g}", bufs=3, space="PSUM"))
            for g in range(2)]
        bootE_sb = consts.tile([_P, bcols], bf16, name="bootE_sb")
        nc.sync.dma_start(bootE_sb[:, :], bootE)            # SP first
        b0f_sb = consts.tile([_P, 3 * _FREE], bf16, name="b0f_sb")
        nc.scalar.dma_start(b0f_sb[:, :], b0f)              # Act queue
        b1_sb = consts.tile([_P, (_NB1 - 3) * _FREE], bf16, name="b1_sb")
        nc.gpsimd.dma_start(b1_sb[:, :], bootb1)            # Pool queue
        b2a_sb = consts.tile([_P, (_NB2A - _NB1) * _FREE], bf16,
                             name="b2a_sb")
        nc.sync.dma_start(b2a_sb[:, :], bootb2a)            # SP second
        b2b_sb = consts.tile([_P, (_NBOOT - _NB2A) * _FREE], bf16,
                             name="b2b_sb")
        nc.sync.dma_start(b2b_sb[:, :], bootb2b)            # SP third

        E2_sb = bootE_sb[:, 0:_P]
        p0 = bootE_sb[:, _P:_P + _FREE]

        hist = consts.tile([_P, (_RLOC + 1) * _FREE], bf16, name="hist")

        chunks = [(27, 34), (35, 50), (51, 65)]
        chunk_iter = iter(chunks)
        state = {"l0": None, "e": None}

        def fsrc(l):
            """F tile (exp'd, bf16) for local step l, from boot or chunk."""
            if l <= 3:
                return b0f_sb[:, (l - 1) * _FREE:l * _FREE]
            if l <= _NB1:
                off = (l - 4) * _FREE
                return b1_sb[:, off:off + _FREE]
            if l <= _NB2A:
                off = (l - _NB1 - 1) * _FREE
                return b2a_sb[:, off:off + _FREE]
            if l <= _NBOOT:
                off = (l - _NB2A - 1) * _FREE
                return b2b_sb[:, off:off + _FREE]
            off = (l - state["l0"]) * _FREE
            return state["e"][:, off:off + _FREE]

        next_chunk = next(chunk_iter)
        curs = [p0[:, g * _CW:(g + 1) * _CW] for g in range(2)]
        # archive blocks: last_slot -> (engine, first_slot)
        arch_plan = {24: ('pool', 1), 48: ('act', 25), 58: ('sp', 49),
                     63: ('pool', 59), 65: ('sp', 64)}

        for l in range(1, _RLOC + 1):
            if next_chunk is not None and l == next_chunk[0]:
                lo, hi = next_chunk
                n = hi - lo + 1
                fch = consts.tile([_P, 16 * _FREE], bf16,
                                  name=f"fch{lo}")
                nc.sync.dma_start(fch[:, :n * _FREE],
                                  featR[:, (lo - _NBOOT - 1) * _FREE:
                                        (hi - _NBOOT) * _FREE])
                state["l0"], state["e"] = lo, fch
                next_chunk = next(chunk_iter, None)
            f_l = fsrc(l)
            for g in range(2):
                fsl = f_l[:, g * _CW:(g + 1) * _CW]
                ps = ps_g[g].tile([_P, _CW], f32, tag=f"ps{g}")
                nc.tensor.matmul(ps[:, :], E2_sb, curs[g],
                                 start=True, stop=True)
                dst = hist[:, l * _FREE + g * _CW:
                           l * _FREE + (g + 1) * _CW]
                nc.vector.tensor_mul(dst, ps[:, :], fsl)
                curs[g] = dst
            if l in arch_plan:
                eng_name, lo = arch_plan[l]
                eng = {'pool': nc.gpsimd, 'act': nc.scalar,
                       'sp': nc.sync}[eng_name]
                eng.dma_start(histo[:, lo * _FREE:(l + 1) * _FREE],
                              hist[:, lo * _FREE:(l + 1) * _FREE])
    nc.compile()
    return nc


def _prep_inputs(feas, transitions):
    import ml_dtypes
    bf = ml_dtypes.bfloat16

    E = np.exp(transitions.astype(np.float32))
    rows = np.ones(_T, bool)
    rows[_START] = False
    c = float(np.log(E.sum(1)[rows]).mean())
    ET = np.ascontiguousarray(E.T).astype(np.float32)       # ET[j,i]=E[i,j]
    E2T = np.zeros((_P, _P), np.float32)
    E2T[:_T, :_T] = ET
    E2T[_T:, _T:] = ET
    # stacked raw features per local step: stk[p, l, n] =
    #   feat[p%64, base+l-1, (p//64)*256 + n] - c   (pad -c past S)
    ft = np.transpose(feas.astype(np.float32), (2, 1, 0)) - np.float32(c)
    # ft: [T, S, B]
    in_maps = []
    for cix in range(_NCORE):
        base = 64 * cix
        stk = np.full((_P, _RLOC + 1, _FREE), -c, np.float32)
        n_real = min(_S - base, _RLOC)              # locals with real feats
        sl = ft[:, base:base + n_real, :]           # [T, n, B]
        stk[:_T, 1:n_real + 1, :] = np.ascontiguousarray(
            sl[:, :, 0:_FREE])
        stk[_T:, 1:n_real + 1, :] = np.ascontiguousarray(
            sl[:, :, _FREE:2 * _FREE])
        expF = np.exp(stk.astype(bf).astype(np.float32)).astype(bf)
        if cix == 0:
            p0 = np.zeros((_P, _FREE), np.float32)
            p0[_START, :] = 1.0
            p0[_T + _START, :] = 1.0
        else:
            p0 = np.full((_P, _FREE), 1.0 / _T, np.float32)
        bootE = np.hstack([E2T, p0]).astype(bf)
        in_maps.append({
            "bootE": np.ascontiguousarray(bootE),
            "b0f": np.ascontiguousarray(
                expF[:, 1:4, :].reshape(_P, -1)),
            "bootb1": np.ascontiguousarray(
                expF[:, 4:_NB1 + 1, :].reshape(_P, -1)),
            "bootb2a": np.ascontiguousarray(
                expF[:, _NB1 + 1:_NB2A + 1, :].reshape(_P, -1)),
            "bootb2b": np.ascontiguousarray(
                expF[:, _NB2A + 1:_NBOOT + 1, :].reshape(_P, -1)),
            "featR": np.ascontiguousarray(
                expF[:, _NBOOT + 1:, :].reshape(_P, -1)),
        })
    return c, in_maps


def kernel(feas, transitions, tag, seq_len):
    from concourse.bass_utils import run_bass_kernel_spmd

    feas = np.asarray(feas)
    transitions = np.asarray(transitions)
    tag = np.asarray(tag)
    seq_len = np.asarray(seq_len)

    if "nc" not in _cache:
        _cache["nc"] = _build_nc()
    nc = _cache["nc"]

    c, in_maps = _prep_inputs(feas, transitions)
    res = run_bass_kernel_spmd(nc, in_maps, list(range(_NCORE))).results

    # ---- host epilogue: telescoped norm from per-core archives ----
    L = seq_len.astype(np.int64)                                      # [B]
    # stops[j, l, b]: archived STOP value; col b -> (row 63 | 127, n=b%256)
    stops = np.zeros((_NCORE, _RLOC + 1, _B))
    s_start = np.zeros((_NCORE, _B))
    s_end = np.zeros((_NCORE, _B))
    for j in range(_NCORE):
        h = res[j]["hist"].reshape(_P, _RLOC + 1, _FREE).astype(np.float64)
        stops[j, :, 0:_FREE] = h[_STOP]
        stops[j, :, _FREE:2 * _FREE] = h[_T + _STOP]
        s_start[j, 0:_FREE] = h[0:_T, _WARM, :].sum(0)
        s_start[j, _FREE:] = h[_T:, _WARM, :].sum(0)
        s_end[j, 0:_FREE] = h[0:_T, _RLOC, :].sum(0)
        s_end[j, _FREE:] = h[_T:, _RLOC, :].sum(0)

    # block growth: core j covers global steps (64j+WARM, 64j+RLOC]
    growth = np.log(s_end) - np.log(s_start)            # [NCORE, B]
    growth[0] = np.log(s_end[0])                        # core 0: from |p0|=1
    prefix = np.concatenate([np.zeros((1, _B)), np.cumsum(growth, 0)], 0)

    m = L + 1                                           # capture slot
    K = np.where(m <= _RLOC, 0, (m - _RLOC - 1) // 64 + 1)
    lloc = m - 64 * K
    bb = np.arange(_B)
    C_raw = np.log(stops[K, lloc, bb])
    lvalue = np.where(
        K == 0,
        C_raw,
        C_raw + prefix[K, bb] - np.log(s_start[K, bb]),
    )
    featT_val = np.where(
        L < _S,
        feas[bb, np.minimum(L, _S - 1), _STOP].astype(np.float64) - c,
        -c,
    )
    norm = c * L + lvalue - featT_val

    # ---- gold score ----
    dt = np.float32
    pos = np.arange(_S + 2)
    lbl = np.concatenate(
        [np.full((_B, 1), _START, tag.dtype), tag,
         np.full((_B, 1), _STOP, tag.dtype)], axis=1,
    )
    lbl = np.where(pos[None, :] <= L[:, None], lbl, _STOP)
    trn = transitions[lbl[:, 1:], lbl[:, :-1]]
    tmask = (np.arange(_S + 1)[None, :] <= L[:, None]).astype(dt)
    trans_score = (trn.astype(dt) * tmask).sum(1)
    emit = np.take_along_axis(feas, tag[..., None], axis=2)[..., 0]
    emask = (np.arange(_S)[None, :] < L[:, None]).astype(dt)
    emit_score = (emit.astype(dt) * emask).sum(1)

    return (norm - (trans_score + emit_score)).astype(np.float32)
